# revision 10
# baseline (speedup 1.0000x reference)
"""Contrastive queue loss kernel for 8 Trainium2 NeuronCores.

Reference computation (all fp32):
    pos[j,b]    = V[j,b,:] . L[b,:] / T                  (J=2, B=256, F=128)
    qlog[j,b,q] = V[j,b,:] . queue[q,:] / T              (Q=65536)
    denom[j,b]  = log( sum_i exp(pos[j,i]) + sum_q exp(qlog[j,b,q]) )
    loss        = -sum_{j,b} (pos[j,b] - denom[j,b]) / B

Sharding: queue split along Q across 8 cores (8192 rows each); V/L replicated.
Each core emits pos[jb] (raw dot products) and its partial sum_q exp(10*logit)
per jb (inputs are L2-normalized so |logit| <= 1 and exp(10*logit) <= e^10 —
no max-subtraction needed for fp32 safety). Host combines partials with a
tiny (4096-element) logsumexp in float64.
"""

import numpy as np

J, B, F, Q = 2, 256, 128, 65536
NCORES = 8
QC = Q // NCORES          # 8192 queue rows per core
JB = J * B                # 512
INV_T = 10.0
NT = JB // 128            # 4 jb tiles of 128
CHUNK = 1024              # queue rows per chunk (8 blocks of 128)
NCHUNK = QC // CHUNK      # 8

_STATE = {}


def _build(stage=99):
    import concourse.tile as tile
    from concourse import bacc, masks, mybir

    f32 = mybir.dt.float32
    nc = bacc.Bacc("TRN2", target_bir_lowering=False, debug=False,
                   num_devices=NCORES)

    v2_d = nc.dram_tensor("V2", (JB, F), f32, kind="ExternalInput")
    l_d = nc.dram_tensor("L", (B, F), f32, kind="ExternalInput")
    q_d = nc.dram_tensor("queue", (QC, F), f32, kind="ExternalInput")
    # out[0, p, t] = pos_raw[jb = t*128 + p]
    # out[1, p, t] = sum over this core's queue shard of exp(10 * logit[jb, q])
    out_d = nc.dram_tensor("out", (2, 128, NT), f32, kind="ExternalOutput")

    with tile.TileContext(nc) as tc:
        with (
            tc.tile_pool(name="const", bufs=1) as const_pool,
            tc.tile_pool(name="vl", bufs=1) as vl_pool,
            tc.tile_pool(name="qt", bufs=3) as qt_pool,
            tc.tile_pool(name="qts", bufs=4) as qts_pool,
            tc.tile_pool(name="trash", bufs=2) as trash_pool,
            tc.tile_pool(name="res", bufs=1) as res_pool,
            tc.tile_pool(name="pst", bufs=2, space="PSUM") as pst_pool,
            tc.tile_pool(name="pslog", bufs=3, space="PSUM") as pslog_pool,
        ):
            ident = const_pool.tile([128, 128], f32)
            if stage >= 2:
                masks.make_identity(nc, ident[:])
            else:
                nc.vector.memset(ident[:], 0.0)

            # ---- setup: V2T [f=128, jb=512] + pos[jb] ----
            vt_all = vl_pool.tile([128, JB], f32)      # [p, (t f)] natural V2
            nc.sync.dma_start(
                vt_all[:].rearrange("p (t f) -> p t f", f=F),
                v2_d.ap().rearrange("(t p) f -> p t f", p=128))
            lt = vl_pool.tile([128, B], f32)           # [p, (u f)] natural L
            nc.sync.dma_start(
                lt[:].rearrange("p (u f) -> p u f", f=F),
                l_d.ap().rearrange("(u p) f -> p u f", p=128))

            v2t = vl_pool.tile([128, JB], f32)         # [f, jb]
            if stage >= 3:
                pv = pst_pool.tile([128, 512], f32, tag="pst")
                for t in range(NT):
                    nc.tensor.transpose(
                        pv[:, t * 128:(t + 1) * 128],
                        vt_all[:, t * 128:(t + 1) * 128], ident[:])
                nc.vector.tensor_copy(v2t[:], pv[:])
            else:
                nc.vector.tensor_copy(v2t[:], vt_all[:])

            pos_sb = res_pool.tile([128, NT], f32)
            nc.vector.memset(pos_sb[:], 0.0)
            junk = trash_pool.tile([128, 128], f32, tag="junk")
            if stage >= 4:
                for t in range(NT):
                    u = t % (B // 128)
                    nc.vector.tensor_mul(
                        junk[:],
                        vt_all[:, t * 128:(t + 1) * 128],
                        lt[:, u * 128:(u + 1) * 128])
                    nc.vector.tensor_reduce(
                        out=pos_sb[:, t:t + 1], in_=junk[:],
                        axis=mybir.AxisListType.X, op=mybir.AluOpType.add)

            # ---- main loop over queue chunks ----
            # acc[p, t*NCHUNK + c] = partial sum for jb tile t, chunk c
            acc = res_pool.tile([128, NT * NCHUNK], f32)
            nc.vector.memset(acc[:], 0.0)
            qv = q_d.ap().rearrange("(c s p) f -> c p s f", p=128, s=CHUNK // 128)
            for c in range(NCHUNK if stage >= 12 else 0):
                qt = qt_pool.tile([128, CHUNK], f32)
                nc.sync.dma_start(
                    qt[:].rearrange("p (s f) -> p s f", f=F), qv[c])
                if stage < 13:
                    continue
                qts_halves = []
                for h in range(CHUNK // 512):
                    pt = pst_pool.tile([128, 512], f32, tag="pst")
                    for k in range(4):
                        s = h * 4 + k
                        nc.tensor.transpose(
                            pt[:, k * 128:(k + 1) * 128],
                            qt[:, s * 128:(s + 1) * 128], ident[:])
                    qts = qts_pool.tile([128, 512], f32)
                    nc.vector.tensor_copy(qts[:], pt[:])
                    qts_halves.append(qts)
                if stage < 14:
                    continue
                for t in range(NT):
                    lg = pslog_pool.tile([128, CHUNK], f32, tag="pslog")
                    for h, qts in enumerate(qts_halves):
                        nc.tensor.matmul(
                            lg[:, h * 512:(h + 1) * 512],
                            lhsT=v2t[:, t * 128:(t + 1) * 128],
                            rhs=qts[:], start=True, stop=True)
                    if stage < 15:
                        continue
                    tr = trash_pool.tile([128, CHUNK], f32, tag="tr")
                    col = t * NCHUNK + c
                    if stage < 16:
                        nc.scalar.activation(
                            tr[:], lg[:], mybir.ActivationFunctionType.Exp,
                            scale=INV_T)
                        nc.vector.tensor_reduce(
                            out=acc[:, col:col + 1], in_=tr[:],
                            axis=mybir.AxisListType.X, op=mybir.AluOpType.add)
                    else:
                        nc.scalar.activation(
                            tr[:], lg[:], mybir.ActivationFunctionType.Exp,
                            scale=INV_T, accum_out=acc[:, col:col + 1])

            # ---- finalize: reduce partial sums over chunks, DMA out ----
            s_sb = res_pool.tile([128, NT], f32)
            for t in range(NT):
                nc.vector.tensor_reduce(
                    out=s_sb[:, t:t + 1],
                    in_=acc[:, t * NCHUNK:(t + 1) * NCHUNK],
                    axis=mybir.AxisListType.X, op=mybir.AluOpType.add)
            nc.sync.dma_start(out_d.ap()[0], pos_sb[:])
            nc.sync.dma_start(out_d.ap()[1], s_sb[:])

    nc.compile()
    return nc


def _run(in_maps, trace=False, **kwargs):
    from concourse.bass_utils import run_bass_kernel_spmd
    if "nc" not in _STATE:
        _STATE["nc"] = _build()
    return run_bass_kernel_spmd(_STATE["nc"], in_maps, list(range(NCORES)),
                                trace=trace, **kwargs)


def _make_in_maps(V, L, queue):
    V2 = np.ascontiguousarray(
        np.asarray(V, dtype=np.float32).reshape(JB, F))
    Ln = np.ascontiguousarray(np.asarray(L, dtype=np.float32))
    qn = np.asarray(queue, dtype=np.float32).reshape(NCORES, QC, F)
    return [{"V2": V2, "L": Ln, "queue": np.ascontiguousarray(qn[i])}
            for i in range(NCORES)]


def _combine(outs):
    """outs: list of (2, 128, NT) arrays, one per core -> scalar loss."""
    pos_raw = outs[0][0].T.reshape(JB).astype(np.float64)   # jb = t*128 + p
    qsum = np.zeros(JB, dtype=np.float64)
    for o in outs:
        qsum += o[1].T.reshape(JB).astype(np.float64)
    pos_s = INV_T * pos_raw
    batch_sum = np.exp(pos_s).reshape(J, B).sum(axis=1)     # sum_i exp(pos[j,i])
    denom = np.log(np.repeat(batch_sum, B) + qsum)
    loss = -(pos_s.sum() - denom.sum()) / B
    return np.array(loss, dtype=np.float32)


def kernel(V, L, queue):
    res = _run(_make_in_maps(V, L, queue))
    return _combine([res.results[i]["out"] for i in range(NCORES)])


# revision 11
# speedup vs baseline: 1.4537x; 1.4537x over previous
"""Contrastive queue loss kernel for 8 Trainium2 NeuronCores.

Reference computation (all fp32):
    pos[j,b]    = V[j,b,:] . L[b,:] / T                  (J=2, B=256, F=128)
    qlog[j,b,q] = V[j,b,:] . queue[q,:] / T              (Q=65536)
    denom[j,b]  = log( sum_i exp(pos[j,i]) + sum_q exp(qlog[j,b,q]) )
    loss        = -sum_{j,b} (pos[j,b] - denom[j,b]) / B

Sharding: queue split along Q across 8 cores (8192 rows each); V/L replicated.
Each core emits pos[jb] (fp32 dot products) and its partial
sum_q exp(10*logit) per jb; logits come from bf16 matmuls (fp32 PSUM
accumulate). Inputs are L2-normalized so |logit| <= 1 and
exp(10*logit) <= e^10 — no max-subtraction needed for fp32 safety.
Host combines partials with a tiny (4096-element) logsumexp in float64.

Per-core dataflow:
  DMA queue chunk (1024 rows, fp32) -> DVE cast to bf16
  -> PE 128x128 transposes into PSUM -> DVE copy to SBUF (queueT, bf16)
  -> PE matmul against persistent V2T (bf16) -> logits in PSUM (fp32)
  -> ACT exp(10x) with fused free-dim accumulation (accum_out)
  -> DVE reduces partial columns, DMA out [2,128,4].
"""

import numpy as np

J, B, F, Q = 2, 256, 128, 65536
NCORES = 8
QC = Q // NCORES          # 8192 queue rows per core
JB = J * B                # 512
INV_T = 10.0
NT = JB // 128            # 4 jb tiles of 128
CHUNK = 1024              # queue rows per chunk (8 blocks of 128)
NCHUNK = QC // CHUNK      # 8
NSLICE = QC // 512        # 16 rhs slices of 512 q rows
# ACT groups per jb tile: q-extents of the fused exp+accumulate ops.
# 5 x 1536 + 1 x 512 = 8192; PSUM budget: 2 x 3 banks (logits)
# + 2 x 1 bank (bf16 transposes) = 8 banks.
GROUPS = [(0, 3), (3, 3), (6, 3), (9, 3), (12, 3), (15, 1)]
NG = len(GROUPS)

_STATE = {}


def _build():
    import concourse.tile as tile
    from concourse import bacc, masks, mybir

    f32 = mybir.dt.float32
    bf16 = mybir.dt.bfloat16
    nc = bacc.Bacc("TRN2", target_bir_lowering=False, debug=False,
                   num_devices=NCORES)

    v2_d = nc.dram_tensor("V2", (JB, F), f32, kind="ExternalInput")
    l_d = nc.dram_tensor("L", (B, F), f32, kind="ExternalInput")
    q_d = nc.dram_tensor("queue", (QC, F), f32, kind="ExternalInput")
    # out[0, p, t] = pos_raw[jb = t*128 + p]
    # out[1, p, t] = sum over this core's queue shard of exp(10 * logit[jb, q])
    out_d = nc.dram_tensor("out", (2, 128, NT), f32, kind="ExternalOutput")

    with tile.TileContext(nc) as tc:
        with (
            tc.tile_pool(name="const", bufs=1) as const_pool,
            tc.tile_pool(name="vl", bufs=1) as vl_pool,
            tc.tile_pool(name="qt", bufs=3) as qt_pool,
            tc.tile_pool(name="qtb", bufs=3) as qtb_pool,
            tc.tile_pool(name="qts", bufs=6) as qts_pool,
            tc.tile_pool(name="trash", bufs=2) as trash_pool,
            tc.tile_pool(name="res", bufs=1) as res_pool,
            tc.tile_pool(name="pst", bufs=2, space="PSUM") as pst_pool,
            tc.tile_pool(name="pslog", bufs=2, space="PSUM") as pslog_pool,
        ):
            identb = const_pool.tile([128, 128], bf16)
            masks.make_identity(nc, identb[:])

            # ---- setup: V2T [f=128, jb=512] bf16 + pos[jb] fp32 ----
            vt_all = vl_pool.tile([128, JB], f32)      # [p, (t f)] natural V2
            nc.sync.dma_start(
                vt_all[:].rearrange("p (t f) -> p t f", f=F),
                v2_d.ap().rearrange("(t p) f -> p t f", p=128))
            lt = vl_pool.tile([128, B], f32)           # [p, (u f)] natural L
            nc.sync.dma_start(
                lt[:].rearrange("p (u f) -> p u f", f=F),
                l_d.ap().rearrange("(u p) f -> p u f", p=128))

            vtb = vl_pool.tile([128, JB], bf16)
            nc.vector.tensor_copy(vtb[:], vt_all[:])
            pv = pst_pool.tile([128, CHUNK], bf16, tag="pst")
            for t in range(NT):
                nc.tensor.transpose(
                    pv[:, t * 128:(t + 1) * 128],
                    vtb[:, t * 128:(t + 1) * 128], identb[:])
            v2tb = vl_pool.tile([128, JB], bf16)       # [f, jb]
            nc.vector.tensor_copy(v2tb[:], pv[:, :JB])

            pos_sb = res_pool.tile([128, NT], f32)
            junk = trash_pool.tile([128, 128], f32, tag="junk")
            for t in range(NT):
                u = t % (B // 128)
                nc.vector.tensor_mul(
                    junk[:],
                    vt_all[:, t * 128:(t + 1) * 128],
                    lt[:, u * 128:(u + 1) * 128])
                nc.vector.tensor_reduce(
                    out=pos_sb[:, t:t + 1], in_=junk[:],
                    axis=mybir.AxisListType.X, op=mybir.AluOpType.add)

            # ---- stream queue chunks: load, cast, transpose ----
            qv = q_d.ap().rearrange("(c s p) f -> c p s f", p=128, s=CHUNK // 128)
            slices = []                                # 16 x [f=128, q=512] bf16
            for c in range(NCHUNK):
                qt = qt_pool.tile([128, CHUNK], f32)
                nc.sync.dma_start(
                    qt[:].rearrange("p (s f) -> p s f", f=F), qv[c])
                qtb = qtb_pool.tile([128, CHUNK], bf16)
                nc.vector.tensor_copy(qtb[:], qt[:])
                pt = pst_pool.tile([128, CHUNK], bf16, tag="pst")
                for s in range(CHUNK // 128):
                    nc.tensor.transpose(
                        pt[:, s * 128:(s + 1) * 128],
                        qtb[:, s * 128:(s + 1) * 128], identb[:])
                qts = qts_pool.tile([128, CHUNK], bf16)
                nc.vector.tensor_copy(qts[:], pt[:])
                slices.append(qts[:, 0:512])
                slices.append(qts[:, 512:CHUNK])

            # ---- logits + fused exp/accumulate ----
            # acc[p, t*NG + g] = partial sum for jb tile t, ACT group g
            acc = res_pool.tile([128, NT * NG], f32)
            for gi, (s0, ns) in enumerate(GROUPS):
                for t in range(NT):
                    lg = pslog_pool.tile([128, 512 * ns], f32, tag="pslog")
                    for k in range(ns):
                        nc.tensor.matmul(
                            lg[:, k * 512:(k + 1) * 512],
                            lhsT=v2tb[:, t * 128:(t + 1) * 128],
                            rhs=slices[s0 + k], start=True, stop=True)
                    tr = trash_pool.tile([128, 512 * ns], f32, tag="tr")
                    col = t * NG + gi
                    nc.scalar.activation(
                        tr[:], lg[:], mybir.ActivationFunctionType.Exp,
                        scale=INV_T, accum_out=acc[:, col:col + 1])

            # ---- finalize: reduce partials over groups, DMA out ----
            s_sb = res_pool.tile([128, NT], f32)
            for t in range(NT):
                nc.vector.tensor_reduce(
                    out=s_sb[:, t:t + 1],
                    in_=acc[:, t * NG:(t + 1) * NG],
                    axis=mybir.AxisListType.X, op=mybir.AluOpType.add)
            nc.sync.dma_start(out_d.ap()[0], pos_sb[:])
            nc.sync.dma_start(out_d.ap()[1], s_sb[:])

    nc.compile()
    return nc


def _run(in_maps, trace=False, **kwargs):
    from concourse.bass_utils import run_bass_kernel_spmd
    if "nc" not in _STATE:
        _STATE["nc"] = _build()
    return run_bass_kernel_spmd(_STATE["nc"], in_maps, list(range(NCORES)),
                                trace=trace, **kwargs)


def _make_in_maps(V, L, queue):
    V2 = np.ascontiguousarray(
        np.asarray(V, dtype=np.float32).reshape(JB, F))
    Ln = np.ascontiguousarray(np.asarray(L, dtype=np.float32))
    qn = np.asarray(queue, dtype=np.float32).reshape(NCORES, QC, F)
    return [{"V2": V2, "L": Ln, "queue": np.ascontiguousarray(qn[i])}
            for i in range(NCORES)]


def _combine(outs):
    """outs: list of (2, 128, NT) arrays, one per core -> scalar loss."""
    pos_raw = outs[0][0].T.reshape(JB).astype(np.float64)   # jb = t*128 + p
    qsum = np.zeros(JB, dtype=np.float64)
    for o in outs:
        qsum += o[1].T.reshape(JB).astype(np.float64)
    pos_s = INV_T * pos_raw
    batch_sum = np.exp(pos_s).reshape(J, B).sum(axis=1)     # sum_i exp(pos[j,i])
    denom = np.log(np.repeat(batch_sum, B) + qsum)
    loss = -(pos_s.sum() - denom.sum()) / B
    return np.array(loss, dtype=np.float32)


def kernel(V, L, queue):
    res = _run(_make_in_maps(V, L, queue))
    return _combine([res.results[i]["out"] for i in range(NCORES)])
